# revision 24
# baseline (speedup 1.0000x reference)
"""Trainium2 Bass kernel for nn_AppearanceDecoder (dense_transformer).

8-core SPMD sharding: core c handles image i=c//2 (of B*T=4) and the
h=c%2 half of its rows.  Conv projections are computed locally (with a
1-row halo supplied by the host-side shard), GroupNorm statistics and the
attention softmax/PV partial sums are combined across each core pair with
tiny 2-rank AllGathers, and the final MLPs run redundantly per pair.

Compute dtypes: conv1x1 in float32r (full-rate fp32), conv3x3 / attention
QK / PV in bf16 with f32 PSUM accumulation, combines and MLPs in
float32r.  GroupNorm is applied exactly:
  - GN1 is applied in place to the conv1x1 output (after a paired stats
    exchange).
  - GN2 is folded into the attention: logits use (a2*E) against the raw
    conv output (the per-query constant cancels in softmax), and the
    final per-channel affine a2*x+b2 is applied to the pooled queries
    (valid because softmax weights sum to 1).
"""

import numpy as np
import ml_dtypes

import concourse.bass as bass
import concourse.tile as tile
from concourse import bacc, mybir
from concourse.bass_utils import run_bass_kernel_spmd

F32 = mybir.dt.float32
F32R = mybir.dt.float32r
BF16 = mybir.dt.bfloat16
F16 = mybir.dt.float16
AX = mybir.AxisListType
OP = mybir.AluOpType
AF = mybir.ActivationFunctionType

B, T, Q, C = 2, 2, 100, 256
NCORES = 8
NPAIR = 2
EPS = 1e-5

# per level: cin, H, W, conv1x1-chunk rows, conv3x3 tile rows
LEVELS = [
    dict(cin=256, H=128, W=128, cr=4, r3=4),
    dict(cin=512, H=64, W=64, cr=8, r3=8),
    dict(cin=1024, H=32, W=32, cr=8, r3=8),
]
for lv in LEVELS:
    lv["hh"] = lv["H"] // 2          # own rows per core
    lv["rs"] = lv["hh"] + 2          # stored rows (pad/halo on both ends)
    lv["d"] = lv["hh"] * lv["W"]     # own pixels per core
    lv["kt"] = lv["cin"] // 128
    # conv1x1 chunks over stored rows
    ch = []
    r0 = 0
    while r0 < lv["rs"]:
        r = min(lv["cr"], lv["rs"] - r0)
        ch.append((r0, r))
        r0 += r
    lv["chunks"] = ch
    lv["n3"] = lv["hh"] // lv["r3"]  # conv3x3 output tiles
    lv["nlg"] = lv["d"] // 512       # logits 512-px tiles
    lv["ndt"] = lv["d"] // 128       # 128-px blocks for PT/FT/PV

REPLICA_GROUPS = [[0, 1], [2, 3], [4, 5], [6, 7]]

_NC_CACHE = {}


def _build():
    nc = bacc.Bacc("TRN2", target_bir_lowering=False, debug=False,
                   num_devices=NCORES)

    # ---- DRAM parameters (per-core inputs) ----
    x_ext = []
    w1t_ext = []
    w2t_ext = []
    b1_ext = []
    b2_ext = []
    gsb_ext = []
    for li, lv in enumerate(LEVELS):
        x_ext.append(nc.declare_dram_parameter(
            f"x{li}", [lv["cin"], lv["rs"], lv["W"]], F32R, isOutput=False))
        w1t_ext.append(nc.declare_dram_parameter(
            f"w1t{li}", [lv["cin"], 256], F32R, isOutput=False))
        w2t_ext.append(nc.declare_dram_parameter(
            f"w2t{li}", [256, 9, 256], F16, isOutput=False))
        b1_ext.append(nc.declare_dram_parameter(
            f"b1_{li}", [256, 1], F32, isOutput=False))
        b2_ext.append(nc.declare_dram_parameter(
            f"b2_{li}", [256, 1], F32, isOutput=False))
        gsb_ext.append(nc.declare_dram_parameter(
            f"gsb{li}", [4, 256, 1], F32, isOutput=False))  # gs1,gb1,gs2,gb2
    embq_ext = nc.declare_dram_parameter("embq", [256, 100], F32, isOutput=False)
    padkeep_ext = nc.declare_dram_parameter("padkeep", [128, 2], F32, isOutput=False)
    g8_ext = nc.declare_dram_parameter("g8", [128, 16], F32R, isOutput=False)
    e16_ext = nc.declare_dram_parameter("e16", [16, 128], F32R, isOutput=False)
    idb_ext = nc.declare_dram_parameter("idb", [128, 128], F16, isOutput=False)
    idr_ext = nc.declare_dram_parameter("idr", [128, 128], F32R, isOutput=False)
    mlpw_names = ["agg1", "agg2", "emb1", "emb2", "trk1", "trk2"]
    mlpw_ext = {}
    mlpb_ext = {}
    for i, nm in enumerate(mlpw_names):
        kdim = 768 if nm == "agg1" else 256
        mlpw_ext[nm] = nc.declare_dram_parameter(
            f"mw_{nm}", [kdim, 256], F32R, isOutput=False)
        mlpb_ext[nm] = nc.declare_dram_parameter(
            f"mb_{nm}", [256, 1], F32, isOutput=False)
    out_ext = nc.declare_dram_parameter("out", [256, 100], F32, isOutput=True)

    # ---- internal DRAM bounce buffers for the pair collectives ----
    gn_in = {}
    gn_out = {}
    for li in range(3):
        for st in (1, 2):
            gn_in[(li, st)] = nc.dram_tensor(f"gnin{li}_{st}", [128, 4], F32)
            gn_out[(li, st)] = nc.dram_tensor(f"gnout{li}_{st}", [256, 4], F32)
    att_in = [nc.dram_tensor(f"attin{li}", [100, 258], F32) for li in range(3)]
    att_out = [nc.dram_tensor(f"attout{li}", [200, 258], F32) for li in range(3)]
    warm_in = nc.dram_tensor("warm_in", [128, 1], F32)
    warm_out = nc.dram_tensor("warm_out", [256, 1], F32)

    with tile.TileContext(nc) as tc:
        with (
            tc.tile_pool(name="sb", bufs=1) as sb,
            tc.tile_pool(name="ps", bufs=1, space="PSUM") as ps,
        ):
            # warm up the ncfw collective path (first collective pays a
            # ~20us cold start; this one overlaps the input DMAs)
            nc.gpsimd.collective_compute(
                "AllGather", OP.bypass,
                ins=[warm_in[:, :]], outs=[warm_out[:, :]],
                replica_groups=REPLICA_GROUPS)

            # ---- persistent small tiles ----
            idb = sb.tile([128, 128], F16, tag="idb")
            nc.sync.dma_start(out=idb, in_=idb_ext[:, :])
            idr = sb.tile([128, 128], F32R, tag="idr")
            nc.sync.dma_start(out=idr, in_=idr_ext[:, :])
            g8 = sb.tile([128, 16], F32R, tag="g8")
            nc.sync.dma_start(out=g8, in_=g8_ext[:, :])
            e16 = sb.tile([16, 128], F32R, tag="e16")
            nc.sync.dma_start(out=e16, in_=e16_ext[:, :])
            pk = sb.tile([128, 2], F32, tag="pk")
            nc.sync.dma_start(out=pk, in_=padkeep_ext[:, :])
            e_sb = sb.tile([128, 2, 100], F32, tag="e_sb")
            nc.sync.dma_start(
                out=e_sb, in_=embq_ext.rearrange("(kt p) q -> p kt q", p=128))
            eps16 = sb.tile([16, 1], F32, tag="eps16")
            nc.vector.memset(eps16, EPS)
            xt = sb.tile([128, 6, 100], F32R, tag="xt")

            # attention buffers; L0 logits get their own tile, while the
            # L1/L2 logits and all transposed-P buffers alias y1 level-0
            # storage (dead after conv3x3_L0, which precedes every attention)
            lg0 = sb.tile([128, 8192], F16, tag="lg")
            lgs = [lg0, None, None]
            ptbs = [None, None, None]

            y1 = []   # per level, per ct: (128, rs, W) f16
            y2 = []   # per level, per ct: (128, hh, W) f16
            st = [dict() for _ in range(3)]
            _wn = [0]

            def pe_warm(n):
                # dummy f16 matmuls to keep the PE HAM un-throttled through
                # the DMA-bound head
                for k in range(n):
                    wp = ps.tile([128, 128], F32, tag="a", bufs=2,
                                 name=f"warm{_wn[0]}")
                    _wn[0] += 1
                    nc.tensor.matmul(wp, lhsT=idb[:, :], rhs=idb[:, :],
                                     start=True, stop=True)

            def conv1x1(li):
                lv = LEVELS[li]
                kt_n, W, rs = lv["kt"], lv["W"], lv["rs"]
                w1sb = sb.tile([128, kt_n, 256], F32R, tag="w1", bufs=1,
                               name=f"w1sb{li}")
                nc.sync.dma_start(
                    out=w1sb,
                    in_=w1t_ext[li].rearrange("(kt p) c -> p kt c", p=128))
                b1sb = sb.tile([128, 2], F32, tag="b1", bufs=3, name=f"b1sb{li}")
                nc.sync.dma_start(
                    out=b1sb, in_=b1_ext[li].rearrange("(mt p) o -> p (mt o)", p=128))
                gsb = sb.tile([128, 4, 2], F32, tag="gsb", bufs=3, name=f"gsb{li}")
                nc.sync.dma_start(
                    out=gsb, in_=gsb_ext[li].rearrange("i (mt p) o -> p i (mt o)", p=128))
                st[li]["gsb"] = gsb
                y1l = []
                for ct in range(2):
                    t = sb.tile([128, rs, W], F16, tag=f"y1_{li}_{ct}",
                                name=f"y1_{li}_{ct}")
                    y1l.append(t)
                y1.append(y1l)
                if li == 0:
                    f0 = y1l[0].rearrange("p a b -> p (a b)")
                    f1 = y1l[1].rearrange("p a b -> p (a b)")
                    ptbs[0] = f0[:, 0:6400].rearrange("p (d q) -> p d q", q=100)
                    ptbs[1] = f1[:, 0:1600].rearrange("p (d q) -> p d q", q=100)
                    ptbs[2] = f1[:, 1600:2000].rearrange("p (d q) -> p d q", q=100)
                    lgs[1] = f1[:, 2048:4096]
                    lgs[2] = f1[:, 4096:4608]
                xr = x_ext[li].rearrange("(kt p) r w -> p kt r w", p=128)
                for ci, (r0, r) in enumerate(lv["chunks"]):
                    xc = sb.tile([128, kt_n, lv["cr"], W], F32R, tag="xc", bufs=3,
                                 name=f"xc{li}_{ci}")
                    nc.sync.dma_start(
                        out=xc[:, :, :r, :], in_=xr[:, :, r0:r0 + r, :])
                    for mt in range(2):
                        cps = ps.tile([128, lv["cr"], W], F32, tag="c", bufs=3,
                                      name=f"c1ps{li}_{ci}_{mt}")
                        for kt in range(kt_n):
                            nc.tensor.matmul(
                                cps[:, :r, :],
                                lhsT=w1sb[:, kt, mt * 128:(mt + 1) * 128],
                                rhs=xc[:, kt, :r, :],
                                start=(kt == 0), stop=(kt == kt_n - 1))
                        nc.scalar.activation(
                            out=y1l[mt][:, r0:r0 + r, :],
                            in_=cps[:, :r, :],
                            func=AF.Identity,
                            bias=b1sb[:, mt:mt + 1], scale=1.0)

            def gn1(li):
                lv = LEVELS[li]
                W, hh = lv["W"], lv["hh"]
                y1f = [y1[li][ct].rearrange("p a b -> p (a b)") for ct in range(2)]
                st[li]["ab1"] = _gn_block(
                    nc, sb, ps, tc, li, 1, lv,
                    [f[:, W:(hh + 1) * W] for f in y1f],
                    g8, e16, eps16, st[li]["gsb"], gn_in, gn_out)

            def apply1(li):
                lv = LEVELS[li]
                rs = lv["rs"]
                ab1 = st[li]["ab1"]
                y1l = y1[li]
                nchk = 4
                bnd = [rs * k // nchk for k in range(nchk + 1)]
                for k in range(nchk):
                    for ct in range(2):
                        nc.scalar.activation(
                            out=y1l[ct][:, bnd[k]:bnd[k + 1], :],
                            in_=y1l[ct][:, bnd[k]:bnd[k + 1], :],
                            func=AF.Identity,
                            bias=ab1[:, ct, 1:2], scale=ab1[:, ct, 0:1])
                        if k == 0:
                            nc.vector.tensor_scalar_mul(
                                out=y1l[ct][:, 0, :], in0=y1l[ct][:, 0, :],
                                scalar1=pk[:, 0:1])
                        if k == nchk - 1:
                            nc.vector.tensor_scalar_mul(
                                out=y1l[ct][:, rs - 1, :],
                                in0=y1l[ct][:, rs - 1, :],
                                scalar1=pk[:, 1:2])

            def conv3x3(li):
                lv = LEVELS[li]
                W, hh, r3, n3 = lv["W"], lv["hh"], lv["r3"], lv["n3"]
                y1l = y1[li]
                w2sb = []
                for ct in range(2):
                    t = sb.tile([128, 9, 256], F16, tag=f"w2_{ct}", bufs=2,
                                name=f"w2sb{li}_{ct}")
                    nc.sync.dma_start(
                        out=t,
                        in_=w2t_ext[li][ct * 128:(ct + 1) * 128, :, :])
                    w2sb.append(t)
                b2sb = sb.tile([128, 2], F32, tag="b2", bufs=3, name=f"b2sb{li}")
                nc.sync.dma_start(
                    out=b2sb, in_=b2_ext[li].rearrange("(mt p) o -> p (mt o)", p=128))
                y2l = []
                for ct in range(2):
                    t = sb.tile([128, hh, W], F16, tag=f"y2_{li}_{ct}",
                                name=f"y2_{li}_{ct}")
                    y2l.append(t)
                y2.append(y2l)
                GRP = 3
                for mt in range(2):
                    for g0 in range(0, n3, GRP):
                        tl = list(range(g0, min(g0 + GRP, n3)))
                        psl = [ps.tile([128, r3, W], F32, tag="c", bufs=3,
                                       name=f"c3ps{li}_{mt}_{t0}") for t0 in tl]
                        idx = 0
                        # dx=1 (full width) first so start=True covers the
                        # whole tile; dx=0/2 accumulate into column subranges
                        # (edge columns keep their implicit zero padding).
                        for kt in range(2):
                            for dy in range(3):
                                for dx in (1, 0, 2):
                                    lhsT = w2sb[kt][:, dy * 3 + dx,
                                                    mt * 128:(mt + 1) * 128]
                                    for ti, t0 in enumerate(tl):
                                        rows = slice(t0 * r3 + dy,
                                                     t0 * r3 + dy + r3)
                                        if dx == 1:
                                            o = psl[ti][:, :, :]
                                            rh = y1l[kt][:, rows, :]
                                        elif dx == 0:
                                            o = psl[ti][:, :, 1:W]
                                            rh = y1l[kt][:, rows, 0:W - 1]
                                        else:
                                            o = psl[ti][:, :, 0:W - 1]
                                            rh = y1l[kt][:, rows, 1:W]
                                        nc.tensor.matmul(
                                            o, lhsT=lhsT, rhs=rh,
                                            start=(idx == 0), stop=(idx == 17))
                                    idx += 1
                        for ti, t0 in enumerate(tl):
                            nc.scalar.activation(
                                out=y2l[mt][:, t0 * r3:(t0 + 1) * r3, :],
                                in_=psl[ti],
                                func=AF.Identity,
                                bias=b2sb[:, mt:mt + 1], scale=1.0)

            def gn2(li):
                lv = LEVELS[li]
                st[li]["ab2"] = _gn_block(
                    nc, sb, ps, tc, li, 2, lv,
                    [y2[li][ct].rearrange("p a b -> p (a b)") for ct in range(2)],
                    g8, e16, eps16, st[li]["gsb"], gn_in, gn_out)

            def attn_a(li):
                st[li]["ag"] = _attn_block_a(
                    nc, sb, ps, tc, li, LEVELS[li], y2[li],
                    st[li]["ab2"], e_sb, lgs[li], ptbs[li], idb, att_in,
                    att_out)

            def attn_b(li):
                _attn_block_b(nc, sb, ps, tc, li, st[li]["ag"],
                              st[li]["ab2"], idr, xt)

            # software-pipelined emission order: conv work for the next level
            # is issued before each GN/attention barrier so the PE never
            # idles waiting on a pair collective.
            pe_warm(24)
            conv1x1(0)
            pe_warm(12)
            gn1(0)
            conv1x1(1)
            pe_warm(12)
            apply1(0)
            gn1(1)
            pe_warm(12)
            conv3x3(0)
            conv1x1(2)
            gn1(2)
            gn2(0)
            apply1(1)
            attn_a(0)
            conv3x3(1)
            apply1(2)
            gn2(1)
            conv3x3(2)
            gn2(2)
            attn_a(1)
            attn_b(0)
            attn_a(2)
            attn_b(1)
            attn_b(2)

            # ---------- MLPs (f32r, redundant per pair) ----------
            mws = {}
            mbs = {}
            for nm in mlpw_names:
                j_n = 6 if nm == "agg1" else 2
                w = sb.tile([128, j_n, 256], F32R, tag=f"mw_{nm}", name=f"mw{nm}")
                nc.sync.dma_start(
                    out=w, in_=mlpw_ext[nm].rearrange("(j p) c -> p j c", p=128))
                mws[nm] = w
                b = sb.tile([128, 2], F32, tag=f"mb_{nm}", name=f"mb{nm}")
                nc.sync.dma_start(
                    out=b, in_=mlpb_ext[nm].rearrange("(mt p) o -> p (mt o)", p=128))
                mbs[nm] = b

            def mlp_layer(in_t, j_n, nm, relu, out_dtype=F32R, name=""):
                o = sb.tile([128, 2, 100], out_dtype, tag="h", bufs=2, name=name)
                for mt in range(2):
                    mp = ps.tile([128, 128], F32, tag="c", bufs=3,
                                 name=f"mp_{nm}_{mt}")
                    for j in range(j_n):
                        nc.tensor.matmul(
                            mp[:, :100],
                            lhsT=mws[nm][:, j, mt * 128:(mt + 1) * 128],
                            rhs=in_t[:, j, :],
                            start=(j == 0), stop=(j == j_n - 1))
                    nc.scalar.activation(
                        out=o[:, mt, :], in_=mp[:, :100],
                        func=AF.Relu if relu else AF.Identity,
                        bias=mbs[nm][:, mt:mt + 1], scale=1.0)
                return o

            h = mlp_layer(xt, 6, "agg1", True, name="h_a1")
            h = mlp_layer(h, 2, "agg2", False, name="h_a2")
            h = mlp_layer(h, 2, "emb1", True, name="h_e1")
            h = mlp_layer(h, 2, "emb2", False, name="h_e2")
            h = mlp_layer(h, 2, "trk1", True, name="h_t1")
            h = mlp_layer(h, 2, "trk2", False, out_dtype=F32, name="h_t2")
            for mt in range(2):
                nc.sync.dma_start(
                    out=out_ext[mt * 128:(mt + 1) * 128, :], in_=h[:, mt, :])

    nc.compile()
    return nc


def _gn_block(nc, sb, ps, tc, li, stage, lv, own_aps, g8, e16, eps16, gsb,
              gn_in, gn_out):
    """Stats over this core's own pixels, pair AllGather, per-channel a/b.

    own_aps: per ct, flat AP (128, npix) of own pixels (npix % 512 == 0).
    Returns ab tile (128, 2, 2): ab[:, ct, 0]=a, ab[:, ct, 1]=b.
    """
    npix = own_aps[0].shape[1]
    nch = npix // 512
    vt = sb.tile([128, 2, 2], F32, tag="vt", bufs=2, name=f"vt{li}_{stage}")
    for ct in range(2):
        stt = sb.tile([128, 16, 6], F32, tag="stt", bufs=2,
                      name=f"stt{li}_{stage}_{ct}")
        for i in range(nch):
            nc.vector.bn_stats(
                out=stt[:, i, :],
                in_=own_aps[ct][:, i * 512:(i + 1) * 512])
        mv = sb.tile([128, 2], F32, tag="mv", bufs=2, name=f"mv{li}_{stage}_{ct}")
        nc.vector.bn_aggr(out=mv, in_=stt[:, :nch, :])
        nc.vector.tensor_copy(vt[:, ct, 0:1], mv[:, 0:1])
        nc.vector.tensor_mul(vt[:, ct, 1:2], mv[:, 0:1], mv[:, 0:1])
        nc.vector.tensor_add(vt[:, ct, 1:2], vt[:, ct, 1:2], mv[:, 1:2])
    nc.gpsimd.dma_start(out=gn_in[(li, stage)][:, :],
                        in_=vt.rearrange("p a b -> p (a b)"))
    nc.gpsimd.collective_compute(
        "AllGather", OP.bypass,
        ins=[gn_in[(li, stage)][:, :]],
        outs=[gn_out[(li, stage)][:, :]],
        replica_groups=REPLICA_GROUPS)
    vg = sb.tile([128, 2, 2, 2], F32, tag="vg", bufs=2, name=f"vg{li}_{stage}")
    # vg[ch, ct, core, stat] <- gn_out[(core*128+ch), 2*ct+stat]
    nc.gpsimd.dma_start(
        out=vg,
        in_=bass.AP(
            tensor=gn_out[(li, stage)].ap().tensor,
            offset=0,
            ap=[[4, 128], [2, 2], [512, 2], [1, 2]]))
    vc = sb.tile([128, 2, 2], F32R, tag="vc", bufs=2, name=f"vc{li}_{stage}")
    nc.vector.tensor_add(vc, vg[:, :, 0, :], vg[:, :, 1, :])

    ab = sb.tile([128, 2, 2], F32, tag=f"ab{stage}", bufs=2,
                 name=f"ab{li}_{stage}")
    for ct in range(2):
        gps = ps.tile([128, 512], F32, tag="a", bufs=2, name=f"gps{li}_{stage}_{ct}")
        nc.tensor.matmul(gps[:16, :2], lhsT=g8[:, :], rhs=vc[:, ct, :],
                         start=True, stop=True)
        gsb16 = sb.tile([16, 4], F32, tag="gsb16", bufs=2,
                        name=f"g16_{li}_{stage}_{ct}")
        nc.vector.tensor_copy(gsb16[:, 0:2], gps[:16, :2])
        # var = msq - m^2 ; rstd = 1/sqrt(var+eps)
        nc.vector.tensor_mul(gsb16[:, 2:3], gsb16[:, 0:1], gsb16[:, 0:1])
        nc.vector.tensor_tensor(
            out=gsb16[:, 2:3], in0=gsb16[:, 1:2], in1=gsb16[:, 2:3],
            op=OP.subtract)
        nc.scalar.activation(out=gsb16[:, 3:4], in_=gsb16[:, 2:3],
                             func=AF.Sqrt, bias=eps16[:, :], scale=1.0)
        nc.vector.reciprocal(gsb16[:, 3:4], gsb16[:, 3:4])
        # expand groups -> channels: (16,2) [m, rstd] @ e16 -> (128,2)
        exin = sb.tile([16, 2], F32R, tag="exin", bufs=2,
                       name=f"exin{li}_{stage}_{ct}")
        nc.vector.tensor_copy(exin[:, 0:1], gsb16[:, 0:1])
        nc.vector.tensor_copy(exin[:, 1:2], gsb16[:, 3:4])
        eps_ = ps.tile([128, 512], F32, tag="a", bufs=2,
                       name=f"eps{li}_{stage}_{ct}")
        nc.tensor.matmul(eps_[:, :2], lhsT=e16[:, :], rhs=exin[:, :],
                         start=True, stop=True)
        mrs = sb.tile([128, 2], F32, tag="mrs", bufs=2,
                      name=f"mrs{li}_{stage}_{ct}")
        nc.vector.tensor_copy(mrs, eps_[:, :2])
        # a = gs * rstd ; b = gb - m * a
        gidx = 0 if stage == 1 else 2
        nc.vector.tensor_mul(ab[:, ct, 0:1], gsb[:, gidx, ct:ct + 1],
                             mrs[:, 1:2])
        tmpb = sb.tile([128, 1], F32, tag="tmpb", bufs=2,
                       name=f"tmpb{li}_{stage}_{ct}")
        nc.vector.tensor_mul(tmpb, mrs[:, 0:1], ab[:, ct, 0:1])
        nc.vector.tensor_tensor(
            out=ab[:, ct, 1:2], in0=gsb[:, gidx + 1, ct:ct + 1], in1=tmpb,
            op=OP.subtract)
    return ab


def _attn_block_a(nc, sb, ps, tc, li, lv, y2l, ab2, e_sb, lg, ptb, idb,
                  att_in, att_out):
    W, hh, r3 = lv["W"], lv["hh"], lv["r3"]
    D, nlg, ndt = lv["d"], lv["nlg"], lv["ndt"]
    rows_lg = 512 // W

    # E' = a2 * E (bf16)
    ep = sb.tile([128, 2, 100], F16, tag="ep", bufs=2, name=f"ep{li}")
    for ct in range(2):
        nc.vector.tensor_scalar_mul(
            out=ep[:, ct, :], in0=e_sb[:, ct, :], scalar1=ab2[:, ct, 0:1])

    att = sb.tile([128, 258], F32, tag="att", bufs=2, name=f"att{li}")
    mx = sb.tile([128, 16], F32, tag="mx", bufs=2, name=f"mx{li}")

    # logits tiles, stored f16 shifted by the per-tile max (keeps the
    # near-max entries at full f16 precision)
    mxn = sb.tile([128, 16], F32, tag="mxn", bufs=2, name=f"mxn{li}")
    for nt in range(nlg):
        lps = ps.tile([128, 512], F32, tag="a", bufs=2, name=f"lps{li}_{nt}")
        for kt in range(2):
            nc.tensor.matmul(
                lps[:100, :],
                lhsT=ep[:, kt, :],
                rhs=y2l[kt][:, nt * rows_lg:(nt + 1) * rows_lg, :],
                start=(kt == 0), stop=(kt == 1))
        nc.vector.tensor_reduce(
            out=mx[:100, nt:nt + 1], in_=lps[:100, :], axis=AX.X, op=OP.max)
        nc.vector.tensor_scalar_mul(
            out=mxn[:100, nt:nt + 1], in0=mx[:100, nt:nt + 1], scalar1=-1.0)
        nc.scalar.activation(
            out=lg[:100, nt * 512:(nt + 1) * 512], in_=lps[:100, :],
            func=AF.Identity, bias=mxn[:100, nt:nt + 1], scale=1.0)
    nc.vector.tensor_reduce(
        out=att[:100, 257:258], in_=mx[:100, :nlg], axis=AX.X, op=OP.max)
    # per-tile exp bias: mx_nt - m
    shf = sb.tile([128, 16], F32, tag="shf", bufs=2, name=f"shf{li}")
    nc.vector.tensor_scalar(
        out=shf[:100, :nlg], in0=mx[:100, :nlg],
        scalar1=att[:100, 257:258], scalar2=None, op0=OP.subtract)
    zac = sb.tile([128, 16], F32, tag="zac", bufs=2, name=f"zac{li}")
    # P = exp(lg + (mx_nt - m)) per tile, transposed immediately into ptb
    for nt in range(nlg):
        pch = sb.tile([128, 512], F16, tag="pch", bufs=3, name=f"pch{li}_{nt}")
        nc.scalar.activation(
            out=pch[:100, :], in_=lg[:100, nt * 512:(nt + 1) * 512],
            func=AF.Exp, bias=shf[:100, nt:nt + 1], scale=1.0,
            accum_out=zac[:100, nt:nt + 1])
        for b in range(4):
            pp = ps.tile([128, 256], F16, tag="a", bufs=2,
                         name=f"pp{li}_{nt}_{b}")
            nc.tensor.transpose(
                pp[:, :100], pch[:100, b * 128:(b + 1) * 128], idb[:100, :100])
            nc.vector.tensor_copy(ptb[:, nt * 4 + b, :], pp[:, :100])
    nc.vector.tensor_reduce(
        out=att[:100, 256:257], in_=zac[:100, :nlg], axis=AX.X, op=OP.add)

    # PV: transpose 128-px blocks of y2, accumulate P^T-weighted sums
    rb = 128 // W if W < 128 else 1  # y2 rows per 128-px block
    pvp = ps.tile([128, 256], F32, tag="pv" if li == 0 else "c",
                  bufs=1 if li == 0 else 3, name=f"pvp{li}")
    for d in range(ndt):
        fp = ps.tile([128, 2, 128], F16, tag="ft", bufs=2, name=f"fp{li}_{d}")
        for ct in range(2):
            if W == 128:
                src = y2l[ct][:, d, :]
            else:
                src = y2l[ct][:, d * rb:(d + 1) * rb, :]
            nc.tensor.transpose(fp[:, ct, :], src, idb[:, :])
        fsb = sb.tile([128, 256], F16, tag="fsb", bufs=4, name=f"fsb{li}_{d}")
        nc.scalar.copy(fsb, fp)
        nc.tensor.matmul(
            pvp[:100, :], lhsT=ptb[:, d, :], rhs=fsb[:, :],
            start=(d == 0), stop=(d == ndt - 1))
    nc.scalar.copy(att[:100, 0:256], pvp[:100, :])

    # pair exchange of [N, Z, m]
    nc.sync.dma_start(out=att_in[li][:, :], in_=att[:100, :])
    nc.gpsimd.collective_compute(
        "AllGather", OP.bypass,
        ins=[att_in[li][:, :]], outs=[att_out[li][:, :]],
        replica_groups=REPLICA_GROUPS)
    ag = sb.tile([128, 2, 258], F32, tag="ag", bufs=2, name=f"ag{li}")
    nc.sync.dma_start(
        out=ag[:100, :, :],
        in_=bass.AP(
            tensor=att_out[li].ap().tensor,
            offset=0,
            ap=[[258, 100], [25800, 2], [1, 258]]))
    return ag


def _attn_block_b(nc, sb, ps, tc, li, ag, ab2, idr, xt):
    # combine: out = (N0*s0 + N1*s1) / (Z0*s0 + Z1*s1), s_c = exp(m_c - M)
    mxp = sb.tile([128, 1], F32, tag="mxp", bufs=2, name=f"mxp{li}")
    nc.vector.tensor_max(mxp[:100, :], ag[:100, 0, 257:258], ag[:100, 1, 257:258])
    negM = sb.tile([128, 1], F32, tag="negM", bufs=2, name=f"negM{li}")
    nc.scalar.mul(negM[:100, :], mxp[:100, :], -1.0)
    s2 = sb.tile([128, 2], F32, tag="s2", bufs=2, name=f"s2_{li}")
    nc.scalar.activation(out=s2[:100, 0:1], in_=ag[:100, 0, 257:258],
                         func=AF.Exp, bias=negM[:100, :], scale=1.0)
    nc.scalar.activation(out=s2[:100, 1:2], in_=ag[:100, 1, 257:258],
                         func=AF.Exp, bias=negM[:100, :], scale=1.0)
    nm = sb.tile([128, 257], F32, tag="nm", bufs=2, name=f"nm{li}")
    nc.vector.tensor_scalar_mul(
        out=nm[:100, :], in0=ag[:100, 0, 0:257], scalar1=s2[:100, 0:1])
    nc.vector.scalar_tensor_tensor(
        out=nm[:100, :], in0=ag[:100, 1, 0:257], scalar=s2[:100, 1:2],
        in1=nm[:100, :], op0=OP.mult, op1=OP.add)
    rden = sb.tile([128, 1], F32, tag="rden", bufs=2, name=f"rden{li}")
    nc.vector.reciprocal(rden[:100, :], nm[:100, 256:257])
    xa = sb.tile([128, 256], F32R, tag="xa", bufs=2, name=f"xa{li}")
    nc.vector.tensor_scalar_mul(
        out=xa[:100, :], in0=nm[:100, 0:256], scalar1=rden[:100, :])
    # transpose to (C, Q) and apply GN2 affine a2*x+b2
    for ct in range(2):
        tp = ps.tile([128, 128], F32R, tag="ft", bufs=2, name=f"tp{li}_{ct}")
        nc.tensor.transpose(
            tp[:, :100], xa[:100, ct * 128:(ct + 1) * 128], idr[:100, :100])
        nc.scalar.activation(
            out=xt[:, li * 2 + ct, :], in_=tp[:, :100],
            func=AF.Identity,
            bias=ab2[:, ct, 1:2], scale=ab2[:, ct, 0:1])


def _get_nc():
    if "nc" not in _NC_CACHE:
        _NC_CACHE["nc"] = _build()
    return _NC_CACHE["nc"]


def _make_in_maps(pred_embds, feat0, feat1, feat2, proj_params, agg_params,
                  emb_params, trk_params):
    feats = [np.asarray(feat0), np.asarray(feat1), np.asarray(feat2)]
    pred_embds = np.asarray(pred_embds)

    common = {}
    for li, lv in enumerate(LEVELS):
        w1, b1, gs1, gb1, w2, b2, gs2, gb2 = [np.asarray(p) for p in proj_params[li]]
        common[f"w1t{li}"] = np.ascontiguousarray(
            w1[:, :, 0, 0].T.astype(np.float32))
        common[f"w2t{li}"] = np.ascontiguousarray(
            w2.transpose(1, 2, 3, 0).reshape(256, 9, 256)).astype(np.float16)
        common[f"b1_{li}"] = b1.reshape(256, 1).astype(np.float32)
        common[f"b2_{li}"] = b2.reshape(256, 1).astype(np.float32)
        common[f"gsb{li}"] = np.stack(
            [gs1.reshape(256, 1), gb1.reshape(256, 1),
             gs2.reshape(256, 1), gb2.reshape(256, 1)]).astype(np.float32)
    mlp_params = {"agg": agg_params, "emb": emb_params, "trk": trk_params}
    for nm, p in mlp_params.items():
        w1, b1, w2, b2 = [np.asarray(q) for q in p]
        common[f"mw_{nm}1"] = np.ascontiguousarray(w1.T.astype(np.float32))
        common[f"mb_{nm}1"] = b1.reshape(256, 1).astype(np.float32)
        common[f"mw_{nm}2"] = np.ascontiguousarray(w2.T.astype(np.float32))
        common[f"mb_{nm}2"] = b2.reshape(256, 1).astype(np.float32)
    common["g8"] = np.kron(np.eye(16, dtype=np.float32),
                           np.ones((8, 1), np.float32)) / 16.0
    common["e16"] = np.kron(np.eye(16, dtype=np.float32),
                            np.ones((1, 8), np.float32))
    common["idb"] = np.eye(128, dtype=np.float16)
    common["idr"] = np.eye(128, dtype=np.float32)

    in_maps = []
    for c in range(NCORES):
        img, h = c // 2, c % 2
        b, t = img // T, img % T
        m = dict(common)
        for li, lv in enumerate(LEVELS):
            H, W, rs, hh = lv["H"], lv["W"], lv["rs"], lv["hh"]
            xs = np.zeros((lv["cin"], rs, W), np.float32)
            start = h * hh - 1
            lo = max(0, start)
            hi = min(H, start + rs)
            xs[:, lo - start:hi - start, :] = feats[li][img, :, lo:hi, :]
            m[f"x{li}"] = xs
        m["embq"] = np.ascontiguousarray(pred_embds[b, :, t, :]).astype(np.float32)
        keep = np.empty((128, 2), np.float32)
        keep[:, 0] = 0.0 if h == 0 else 1.0   # keep top row? (0 => zero it)
        keep[:, 1] = 1.0 if h == 0 else 0.0   # keep bottom row?
        m["padkeep"] = keep
        in_maps.append(m)
    return in_maps


def kernel(pred_embds, feat0, feat1, feat2, proj_params, agg_params,
           emb_params, trk_params):
    in_maps = _make_in_maps(pred_embds, feat0, feat1, feat2, proj_params,
                            agg_params, emb_params, trk_params)
    nc = _get_nc()
    res = run_bass_kernel_spmd(nc, in_maps, core_ids=list(range(NCORES)))
    out = np.empty((B, T, Q, C), np.float32)
    for img in range(B * T):
        b, t = img // T, img % T
        out[b, t] = res.results[2 * img]["out"].T
    return out


# revision 25
# speedup vs baseline: 1.1249x; 1.1249x over previous
"""Trainium2 Bass kernel for nn_AppearanceDecoder (dense_transformer).

8-core SPMD sharding: core c handles image i=c//2 (of B*T=4) and the
h=c%2 half of its rows.  Conv projections are computed locally (with a
1-row halo supplied by the host-side shard), GroupNorm statistics and the
attention softmax/PV partial sums are combined across each core pair with
tiny 2-rank AllGathers, and the final MLPs run redundantly per pair.

Compute dtypes: conv1x1 in float32r (full-rate fp32), conv3x3 / attention
QK / PV in bf16 with f32 PSUM accumulation, combines and MLPs in
float32r.  GroupNorm is applied exactly:
  - GN1 is applied in place to the conv1x1 output (after a paired stats
    exchange).
  - GN2 is folded into the attention: logits use (a2*E) against the raw
    conv output (the per-query constant cancels in softmax), and the
    final per-channel affine a2*x+b2 is applied to the pooled queries
    (valid because softmax weights sum to 1).
"""

import numpy as np
import ml_dtypes

import concourse.bass as bass
import concourse.tile as tile
from concourse import bacc, mybir
from concourse.bass_utils import run_bass_kernel_spmd

F32 = mybir.dt.float32
F32R = mybir.dt.float32r
BF16 = mybir.dt.bfloat16
F16 = mybir.dt.float16
AX = mybir.AxisListType
OP = mybir.AluOpType
AF = mybir.ActivationFunctionType

B, T, Q, C = 2, 2, 100, 256
NCORES = 8
NPAIR = 2
EPS = 1e-5

# per level: cin, H, W, conv1x1-chunk rows, conv3x3 tile rows
LEVELS = [
    dict(cin=256, H=128, W=128, cr=4, r3=4),
    dict(cin=512, H=64, W=64, cr=8, r3=8),
    dict(cin=1024, H=32, W=32, cr=8, r3=8),
]
for lv in LEVELS:
    lv["hh"] = lv["H"] // 2          # own rows per core
    lv["rs"] = lv["hh"] + 2          # stored rows (pad/halo on both ends)
    lv["d"] = lv["hh"] * lv["W"]     # own pixels per core
    lv["kt"] = lv["cin"] // 128
    # conv1x1 chunks over stored rows
    ch = []
    r0 = 0
    while r0 < lv["rs"]:
        r = min(lv["cr"], lv["rs"] - r0)
        ch.append((r0, r))
        r0 += r
    lv["chunks"] = ch
    lv["n3"] = lv["hh"] // lv["r3"]  # conv3x3 output tiles
    lv["nlg"] = lv["d"] // 512       # logits 512-px tiles
    lv["ndt"] = lv["d"] // 128       # 128-px blocks for PT/FT/PV

REPLICA_GROUPS = [[0, 1], [2, 3], [4, 5], [6, 7]]

_NC_CACHE = {}


def _build():
    nc = bacc.Bacc("TRN2", target_bir_lowering=False, debug=False,
                   num_devices=NCORES)

    # ---- DRAM parameters (per-core inputs) ----
    x_ext = []
    w1t_ext = []
    w2t_ext = []
    b1_ext = []
    b2_ext = []
    gsb_ext = []
    for li, lv in enumerate(LEVELS):
        x_ext.append(nc.declare_dram_parameter(
            f"x{li}", [lv["cin"], lv["rs"], lv["W"]], F32R, isOutput=False))
        w1t_ext.append(nc.declare_dram_parameter(
            f"w1t{li}", [lv["cin"], 256], F32R, isOutput=False))
        w2t_ext.append(nc.declare_dram_parameter(
            f"w2t{li}", [256, 9, 256], F16, isOutput=False))
        b1_ext.append(nc.declare_dram_parameter(
            f"b1_{li}", [256, 1], F32, isOutput=False))
        b2_ext.append(nc.declare_dram_parameter(
            f"b2_{li}", [256, 1], F32, isOutput=False))
        gsb_ext.append(nc.declare_dram_parameter(
            f"gsb{li}", [4, 256, 1], F32, isOutput=False))  # gs1,gb1,gs2,gb2
    embq_ext = nc.declare_dram_parameter("embq", [256, 100], F32, isOutput=False)
    padkeep_ext = nc.declare_dram_parameter("padkeep", [128, 2], F32, isOutput=False)
    g8_ext = nc.declare_dram_parameter("g8", [128, 16], F32R, isOutput=False)
    e16_ext = nc.declare_dram_parameter("e16", [16, 128], F32R, isOutput=False)
    idb_ext = nc.declare_dram_parameter("idb", [128, 128], F16, isOutput=False)
    idr_ext = nc.declare_dram_parameter("idr", [128, 128], F32R, isOutput=False)
    mlpw_names = ["agg1", "agg2", "emb1", "emb2", "trk1", "trk2"]
    mlpw_ext = {}
    mlpb_ext = {}
    for i, nm in enumerate(mlpw_names):
        kdim = 768 if nm == "agg1" else 256
        mlpw_ext[nm] = nc.declare_dram_parameter(
            f"mw_{nm}", [kdim, 256], F32R, isOutput=False)
        mlpb_ext[nm] = nc.declare_dram_parameter(
            f"mb_{nm}", [256, 1], F32, isOutput=False)
    out_ext = nc.declare_dram_parameter("out", [256, 100], F32, isOutput=True)

    # ---- internal DRAM bounce buffers for the pair collectives ----
    gn_in = {}
    gn_out = {}
    for li in range(3):
        for st in (1, 2):
            gn_in[(li, st)] = nc.dram_tensor(f"gnin{li}_{st}", [128, 4], F32)
            gn_out[(li, st)] = nc.dram_tensor(f"gnout{li}_{st}", [256, 4], F32)
    att_in = [nc.dram_tensor(f"attin{li}", [100, 258], F32) for li in range(3)]
    att_out = [nc.dram_tensor(f"attout{li}", [200, 258], F32) for li in range(3)]
    warm_in = nc.dram_tensor("warm_in", [128, 1], F32)
    warm_out = nc.dram_tensor("warm_out", [256, 1], F32)

    with tile.TileContext(nc) as tc:
        with (
            tc.tile_pool(name="sb", bufs=1) as sb,
            tc.tile_pool(name="ps", bufs=1, space="PSUM") as ps,
        ):
            # warm up the ncfw collective path (first collective pays a
            # ~20us cold start; this one overlaps the input DMAs)
            nc.gpsimd.collective_compute(
                "AllGather", OP.bypass,
                ins=[warm_in[:, :]], outs=[warm_out[:, :]],
                replica_groups=REPLICA_GROUPS)

            # ---- persistent small tiles ----
            idb = sb.tile([128, 128], F16, tag="idb")
            nc.sync.dma_start(out=idb, in_=idb_ext[:, :])
            idr = sb.tile([128, 128], F32R, tag="idr")
            nc.sync.dma_start(out=idr, in_=idr_ext[:, :])
            g8 = sb.tile([128, 16], F32R, tag="g8")
            nc.sync.dma_start(out=g8, in_=g8_ext[:, :])
            e16 = sb.tile([16, 128], F32R, tag="e16")
            nc.sync.dma_start(out=e16, in_=e16_ext[:, :])
            pk = sb.tile([128, 2], F32, tag="pk")
            nc.sync.dma_start(out=pk, in_=padkeep_ext[:, :])
            e_sb = sb.tile([128, 2, 100], F32, tag="e_sb")
            nc.sync.dma_start(
                out=e_sb, in_=embq_ext.rearrange("(kt p) q -> p kt q", p=128))
            eps16 = sb.tile([16, 1], F32, tag="eps16")
            nc.vector.memset(eps16, EPS)
            xt = sb.tile([128, 6, 100], F32R, tag="xt")

            # attention buffers; L0 logits get their own tile, while the
            # L1/L2 logits and all transposed-P buffers alias y1 level-0
            # storage (dead after conv3x3_L0, which precedes every attention)
            lg0 = sb.tile([128, 8192], F16, tag="lg")
            lgs = [lg0, None, None]
            ptbs = [None, None, None]

            y1 = []   # per level, per ct: (128, rs, W) f16
            y2 = []   # per level, per ct: (128, hh, W) f16
            st = [dict() for _ in range(3)]
            _wn = [0]

            def pe_warm(n):
                # dummy f16 matmuls to keep the PE HAM un-throttled through
                # the DMA-bound head
                for k in range(n):
                    wp = ps.tile([128, 128], F32, tag="a", bufs=2,
                                 name=f"warm{_wn[0]}")
                    _wn[0] += 1
                    nc.tensor.matmul(wp, lhsT=idb[:, :], rhs=idb[:, :],
                                     start=True, stop=True)

            def conv1x1(li):
                lv = LEVELS[li]
                kt_n, W, rs = lv["kt"], lv["W"], lv["rs"]
                w1sb = sb.tile([128, kt_n, 256], F32R, tag="w1", bufs=1,
                               name=f"w1sb{li}")
                nc.sync.dma_start(
                    out=w1sb,
                    in_=w1t_ext[li].rearrange("(kt p) c -> p kt c", p=128))
                b1sb = sb.tile([128, 2], F32, tag="b1", bufs=3, name=f"b1sb{li}")
                nc.sync.dma_start(
                    out=b1sb, in_=b1_ext[li].rearrange("(mt p) o -> p (mt o)", p=128))
                gsb = sb.tile([128, 4, 2], F32, tag="gsb", bufs=3, name=f"gsb{li}")
                nc.sync.dma_start(
                    out=gsb, in_=gsb_ext[li].rearrange("i (mt p) o -> p i (mt o)", p=128))
                st[li]["gsb"] = gsb
                y1l = []
                for ct in range(2):
                    t = sb.tile([128, rs, W], F16, tag=f"y1_{li}_{ct}",
                                name=f"y1_{li}_{ct}")
                    y1l.append(t)
                y1.append(y1l)
                if li == 0:
                    f0 = y1l[0].rearrange("p a b -> p (a b)")
                    p0 = f0[:, 0:6400].rearrange("p (d q) -> p d q", q=100)
                    ptbs[0] = ptbs[1] = ptbs[2] = p0
                    lgs[1] = lgs[2] = lg0
                xr = x_ext[li].rearrange("(kt p) r w -> p kt r w", p=128)
                for ci, (r0, r) in enumerate(lv["chunks"]):
                    xc = sb.tile([128, kt_n, lv["cr"], W], F32R, tag="xc", bufs=3,
                                 name=f"xc{li}_{ci}")
                    nc.sync.dma_start(
                        out=xc[:, :, :r, :], in_=xr[:, :, r0:r0 + r, :])
                    for mt in range(2):
                        cps = ps.tile([128, lv["cr"], W], F32, tag="c", bufs=3,
                                      name=f"c1ps{li}_{ci}_{mt}")
                        for kt in range(kt_n):
                            nc.tensor.matmul(
                                cps[:, :r, :],
                                lhsT=w1sb[:, kt, mt * 128:(mt + 1) * 128],
                                rhs=xc[:, kt, :r, :],
                                start=(kt == 0), stop=(kt == kt_n - 1))
                        nc.scalar.activation(
                            out=y1l[mt][:, r0:r0 + r, :],
                            in_=cps[:, :r, :],
                            func=AF.Identity,
                            bias=b1sb[:, mt:mt + 1], scale=1.0)

            def gn1(li):
                lv = LEVELS[li]
                W, hh = lv["W"], lv["hh"]
                y1f = [y1[li][ct].rearrange("p a b -> p (a b)") for ct in range(2)]
                st[li]["ab1"] = _gn_block(
                    nc, sb, ps, tc, li, 1, lv,
                    [f[:, W:(hh + 1) * W] for f in y1f],
                    g8, e16, eps16, st[li]["gsb"], gn_in, gn_out)

            def apply1(li):
                lv = LEVELS[li]
                rs = lv["rs"]
                ab1 = st[li]["ab1"]
                y1l = y1[li]
                nchk = 4
                bnd = [rs * k // nchk for k in range(nchk + 1)]
                for k in range(nchk):
                    for ct in range(2):
                        nc.scalar.activation(
                            out=y1l[ct][:, bnd[k]:bnd[k + 1], :],
                            in_=y1l[ct][:, bnd[k]:bnd[k + 1], :],
                            func=AF.Identity,
                            bias=ab1[:, ct, 1:2], scale=ab1[:, ct, 0:1])
                        if k == 0:
                            nc.vector.tensor_scalar_mul(
                                out=y1l[ct][:, 0, :], in0=y1l[ct][:, 0, :],
                                scalar1=pk[:, 0:1])
                        if k == nchk - 1:
                            nc.vector.tensor_scalar_mul(
                                out=y1l[ct][:, rs - 1, :],
                                in0=y1l[ct][:, rs - 1, :],
                                scalar1=pk[:, 1:2])

            def conv3x3(li):
                lv = LEVELS[li]
                W, hh, r3, n3 = lv["W"], lv["hh"], lv["r3"], lv["n3"]
                y1l = y1[li]
                w2sb = []
                for ct in range(2):
                    t = sb.tile([128, 9, 256], F16, tag=f"w2_{ct}", bufs=2,
                                name=f"w2sb{li}_{ct}")
                    nc.sync.dma_start(
                        out=t,
                        in_=w2t_ext[li][ct * 128:(ct + 1) * 128, :, :])
                    w2sb.append(t)
                b2sb = sb.tile([128, 2], F32, tag="b2", bufs=3, name=f"b2sb{li}")
                nc.sync.dma_start(
                    out=b2sb, in_=b2_ext[li].rearrange("(mt p) o -> p (mt o)", p=128))
                y2l = []
                for ct in range(2):
                    t = sb.tile([128, hh, W], F16, tag=f"y2_{li}_{ct}",
                                name=f"y2_{li}_{ct}")
                    y2l.append(t)
                y2.append(y2l)
                GRP = 3
                for mt in range(2):
                    for g0 in range(0, n3, GRP):
                        tl = list(range(g0, min(g0 + GRP, n3)))
                        psl = [ps.tile([128, r3, W], F32, tag="c", bufs=3,
                                       name=f"c3ps{li}_{mt}_{t0}") for t0 in tl]
                        idx = 0
                        # dx=1 (full width) first so start=True covers the
                        # whole tile; dx=0/2 accumulate into column subranges
                        # (edge columns keep their implicit zero padding).
                        for kt in range(2):
                            for dy in range(3):
                                for dx in (1, 0, 2):
                                    lhsT = w2sb[kt][:, dy * 3 + dx,
                                                    mt * 128:(mt + 1) * 128]
                                    for ti, t0 in enumerate(tl):
                                        rows = slice(t0 * r3 + dy,
                                                     t0 * r3 + dy + r3)
                                        if dx == 1:
                                            o = psl[ti][:, :, :]
                                            rh = y1l[kt][:, rows, :]
                                        elif dx == 0:
                                            o = psl[ti][:, :, 1:W]
                                            rh = y1l[kt][:, rows, 0:W - 1]
                                        else:
                                            o = psl[ti][:, :, 0:W - 1]
                                            rh = y1l[kt][:, rows, 1:W]
                                        nc.tensor.matmul(
                                            o, lhsT=lhsT, rhs=rh,
                                            start=(idx == 0), stop=(idx == 17))
                                    idx += 1
                        for ti, t0 in enumerate(tl):
                            nc.scalar.activation(
                                out=y2l[mt][:, t0 * r3:(t0 + 1) * r3, :],
                                in_=psl[ti],
                                func=AF.Identity,
                                bias=b2sb[:, mt:mt + 1], scale=1.0)

            def gn2(li):
                lv = LEVELS[li]
                st[li]["ab2"] = _gn_block(
                    nc, sb, ps, tc, li, 2, lv,
                    [y2[li][ct].rearrange("p a b -> p (a b)") for ct in range(2)],
                    g8, e16, eps16, st[li]["gsb"], gn_in, gn_out)

            def attn_a(li):
                st[li]["ag"] = _attn_block_a(
                    nc, sb, ps, tc, li, LEVELS[li], y2[li],
                    st[li]["ab2"], e_sb, lgs[li], ptbs[li], idb, att_in,
                    att_out)

            def attn_b(li):
                _attn_block_b(nc, sb, ps, tc, li, st[li]["ag"],
                              st[li]["ab2"], idr, xt)

            # software-pipelined emission order: conv work for the next level
            # is issued before each GN/attention barrier so the PE never
            # idles waiting on a pair collective.
            pe_warm(24)
            conv1x1(0)
            pe_warm(12)
            gn1(0)
            conv1x1(1)
            pe_warm(12)
            apply1(0)
            gn1(1)
            pe_warm(12)
            conv3x3(0)
            conv1x1(2)
            gn1(2)
            gn2(0)
            apply1(1)
            attn_a(0)
            attn_b(0)
            conv3x3(1)
            apply1(2)
            gn2(1)
            conv3x3(2)
            gn2(2)
            attn_a(1)
            attn_a(2)
            attn_b(1)
            attn_b(2)

            # ---------- MLPs (f32r, redundant per pair) ----------
            mws = {}
            mbs = {}
            for nm in mlpw_names:
                j_n = 6 if nm == "agg1" else 2
                w = sb.tile([128, j_n, 256], F32R, tag=f"mw_{nm}", name=f"mw{nm}")
                nc.sync.dma_start(
                    out=w, in_=mlpw_ext[nm].rearrange("(j p) c -> p j c", p=128))
                mws[nm] = w
                b = sb.tile([128, 2], F32, tag=f"mb_{nm}", name=f"mb{nm}")
                nc.sync.dma_start(
                    out=b, in_=mlpb_ext[nm].rearrange("(mt p) o -> p (mt o)", p=128))
                mbs[nm] = b

            def mlp_layer(in_t, j_n, nm, relu, out_dtype=F32R, name=""):
                o = sb.tile([128, 2, 100], out_dtype, tag="h", bufs=2, name=name)
                for mt in range(2):
                    mp = ps.tile([128, 128], F32, tag="c", bufs=3,
                                 name=f"mp_{nm}_{mt}")
                    for j in range(j_n):
                        nc.tensor.matmul(
                            mp[:, :100],
                            lhsT=mws[nm][:, j, mt * 128:(mt + 1) * 128],
                            rhs=in_t[:, j, :],
                            start=(j == 0), stop=(j == j_n - 1))
                    nc.scalar.activation(
                        out=o[:, mt, :], in_=mp[:, :100],
                        func=AF.Relu if relu else AF.Identity,
                        bias=mbs[nm][:, mt:mt + 1], scale=1.0)
                return o

            h = mlp_layer(xt, 6, "agg1", True, name="h_a1")
            h = mlp_layer(h, 2, "agg2", False, name="h_a2")
            h = mlp_layer(h, 2, "emb1", True, name="h_e1")
            h = mlp_layer(h, 2, "emb2", False, name="h_e2")
            h = mlp_layer(h, 2, "trk1", True, name="h_t1")
            h = mlp_layer(h, 2, "trk2", False, out_dtype=F32, name="h_t2")
            for mt in range(2):
                nc.sync.dma_start(
                    out=out_ext[mt * 128:(mt + 1) * 128, :], in_=h[:, mt, :])

    nc.compile()
    return nc


def _gn_block(nc, sb, ps, tc, li, stage, lv, own_aps, g8, e16, eps16, gsb,
              gn_in, gn_out):
    """Stats over this core's own pixels, pair AllGather, per-channel a/b.

    own_aps: per ct, flat AP (128, npix) of own pixels (npix % 512 == 0).
    Returns ab tile (128, 2, 2): ab[:, ct, 0]=a, ab[:, ct, 1]=b.
    """
    npix = own_aps[0].shape[1]
    nch = npix // 512
    vt = sb.tile([128, 2, 2], F32, tag="vt", bufs=2, name=f"vt{li}_{stage}")
    for ct in range(2):
        stt = sb.tile([128, 16, 6], F32, tag="stt", bufs=2,
                      name=f"stt{li}_{stage}_{ct}")
        for i in range(nch):
            nc.vector.bn_stats(
                out=stt[:, i, :],
                in_=own_aps[ct][:, i * 512:(i + 1) * 512])
        mv = sb.tile([128, 2], F32, tag="mv", bufs=2, name=f"mv{li}_{stage}_{ct}")
        nc.vector.bn_aggr(out=mv, in_=stt[:, :nch, :])
        nc.vector.tensor_copy(vt[:, ct, 0:1], mv[:, 0:1])
        nc.vector.tensor_mul(vt[:, ct, 1:2], mv[:, 0:1], mv[:, 0:1])
        nc.vector.tensor_add(vt[:, ct, 1:2], vt[:, ct, 1:2], mv[:, 1:2])
    nc.gpsimd.dma_start(out=gn_in[(li, stage)][:, :],
                        in_=vt.rearrange("p a b -> p (a b)"))
    nc.gpsimd.collective_compute(
        "AllGather", OP.bypass,
        ins=[gn_in[(li, stage)][:, :]],
        outs=[gn_out[(li, stage)][:, :]],
        replica_groups=REPLICA_GROUPS)
    vg = sb.tile([128, 2, 2, 2], F32, tag="vg", bufs=2, name=f"vg{li}_{stage}")
    # vg[ch, ct, core, stat] <- gn_out[(core*128+ch), 2*ct+stat]
    nc.gpsimd.dma_start(
        out=vg,
        in_=bass.AP(
            tensor=gn_out[(li, stage)].ap().tensor,
            offset=0,
            ap=[[4, 128], [2, 2], [512, 2], [1, 2]]))
    vc = sb.tile([128, 2, 2], F32R, tag="vc", bufs=2, name=f"vc{li}_{stage}")
    nc.vector.tensor_add(vc, vg[:, :, 0, :], vg[:, :, 1, :])

    ab = sb.tile([128, 2, 2], F32, tag=f"ab{stage}", bufs=2,
                 name=f"ab{li}_{stage}")
    for ct in range(2):
        gps = ps.tile([128, 512], F32, tag="a", bufs=2, name=f"gps{li}_{stage}_{ct}")
        nc.tensor.matmul(gps[:16, :2], lhsT=g8[:, :], rhs=vc[:, ct, :],
                         start=True, stop=True)
        gsb16 = sb.tile([16, 4], F32, tag="gsb16", bufs=2,
                        name=f"g16_{li}_{stage}_{ct}")
        nc.vector.tensor_copy(gsb16[:, 0:2], gps[:16, :2])
        # var = msq - m^2 ; rstd = 1/sqrt(var+eps)
        nc.vector.tensor_mul(gsb16[:, 2:3], gsb16[:, 0:1], gsb16[:, 0:1])
        nc.vector.tensor_tensor(
            out=gsb16[:, 2:3], in0=gsb16[:, 1:2], in1=gsb16[:, 2:3],
            op=OP.subtract)
        nc.scalar.activation(out=gsb16[:, 3:4], in_=gsb16[:, 2:3],
                             func=AF.Sqrt, bias=eps16[:, :], scale=1.0)
        nc.vector.reciprocal(gsb16[:, 3:4], gsb16[:, 3:4])
        # expand groups -> channels: (16,2) [m, rstd] @ e16 -> (128,2)
        exin = sb.tile([16, 2], F32R, tag="exin", bufs=2,
                       name=f"exin{li}_{stage}_{ct}")
        nc.vector.tensor_copy(exin[:, 0:1], gsb16[:, 0:1])
        nc.vector.tensor_copy(exin[:, 1:2], gsb16[:, 3:4])
        eps_ = ps.tile([128, 512], F32, tag="a", bufs=2,
                       name=f"eps{li}_{stage}_{ct}")
        nc.tensor.matmul(eps_[:, :2], lhsT=e16[:, :], rhs=exin[:, :],
                         start=True, stop=True)
        mrs = sb.tile([128, 2], F32, tag="mrs", bufs=2,
                      name=f"mrs{li}_{stage}_{ct}")
        nc.vector.tensor_copy(mrs, eps_[:, :2])
        # a = gs * rstd ; b = gb - m * a
        gidx = 0 if stage == 1 else 2
        nc.vector.tensor_mul(ab[:, ct, 0:1], gsb[:, gidx, ct:ct + 1],
                             mrs[:, 1:2])
        tmpb = sb.tile([128, 1], F32, tag="tmpb", bufs=2,
                       name=f"tmpb{li}_{stage}_{ct}")
        nc.vector.tensor_mul(tmpb, mrs[:, 0:1], ab[:, ct, 0:1])
        nc.vector.tensor_tensor(
            out=ab[:, ct, 1:2], in0=gsb[:, gidx + 1, ct:ct + 1], in1=tmpb,
            op=OP.subtract)
    return ab


def _attn_block_a(nc, sb, ps, tc, li, lv, y2l, ab2, e_sb, lg, ptb, idb,
                  att_in, att_out):
    W, hh, r3 = lv["W"], lv["hh"], lv["r3"]
    D, nlg, ndt = lv["d"], lv["nlg"], lv["ndt"]
    rows_lg = 512 // W

    # E' = a2 * E (bf16)
    ep = sb.tile([128, 2, 100], F16, tag="ep", bufs=2, name=f"ep{li}")
    for ct in range(2):
        nc.vector.tensor_scalar_mul(
            out=ep[:, ct, :], in0=e_sb[:, ct, :], scalar1=ab2[:, ct, 0:1])

    att = sb.tile([128, 258], F32, tag="att", bufs=2, name=f"att{li}")
    mx = sb.tile([128, 16], F32, tag="mx", bufs=2, name=f"mx{li}")

    # logits tiles, stored f16 shifted by the per-tile max (keeps the
    # near-max entries at full f16 precision)
    mxn = sb.tile([128, 16], F32, tag="mxn", bufs=2, name=f"mxn{li}")
    for nt in range(nlg):
        lps = ps.tile([128, 512], F32, tag="a", bufs=2, name=f"lps{li}_{nt}")
        for kt in range(2):
            nc.tensor.matmul(
                lps[:100, :],
                lhsT=ep[:, kt, :],
                rhs=y2l[kt][:, nt * rows_lg:(nt + 1) * rows_lg, :],
                start=(kt == 0), stop=(kt == 1))
        nc.vector.tensor_reduce(
            out=mx[:100, nt:nt + 1], in_=lps[:100, :], axis=AX.X, op=OP.max)
        nc.vector.tensor_scalar_mul(
            out=mxn[:100, nt:nt + 1], in0=mx[:100, nt:nt + 1], scalar1=-1.0)
        nc.scalar.activation(
            out=lg[:100, nt * 512:(nt + 1) * 512], in_=lps[:100, :],
            func=AF.Identity, bias=mxn[:100, nt:nt + 1], scale=1.0)
    nc.vector.tensor_reduce(
        out=att[:100, 257:258], in_=mx[:100, :nlg], axis=AX.X, op=OP.max)
    # per-tile exp bias: mx_nt - m
    shf = sb.tile([128, 16], F32, tag="shf", bufs=2, name=f"shf{li}")
    nc.vector.tensor_scalar(
        out=shf[:100, :nlg], in0=mx[:100, :nlg],
        scalar1=att[:100, 257:258], scalar2=None, op0=OP.subtract)
    zac = sb.tile([128, 16], F32, tag="zac", bufs=2, name=f"zac{li}")
    # P = exp(lg + (mx_nt - m)) per tile, transposed immediately into ptb
    for nt in range(nlg):
        pch = sb.tile([128, 512], F16, tag="pch", bufs=3, name=f"pch{li}_{nt}")
        nc.scalar.activation(
            out=pch[:100, :], in_=lg[:100, nt * 512:(nt + 1) * 512],
            func=AF.Exp, bias=shf[:100, nt:nt + 1], scale=1.0,
            accum_out=zac[:100, nt:nt + 1])
        for b in range(4):
            pp = ps.tile([128, 256], F16, tag="a", bufs=2,
                         name=f"pp{li}_{nt}_{b}")
            nc.tensor.transpose(
                pp[:, :100], pch[:100, b * 128:(b + 1) * 128], idb[:100, :100])
            nc.vector.tensor_copy(ptb[:, nt * 4 + b, :], pp[:, :100])
    nc.vector.tensor_reduce(
        out=att[:100, 256:257], in_=zac[:100, :nlg], axis=AX.X, op=OP.add)

    # PV: transpose 128-px blocks of y2, accumulate P^T-weighted sums
    rb = 128 // W if W < 128 else 1  # y2 rows per 128-px block
    pvp = ps.tile([128, 256], F32, tag="pv", bufs=1, name=f"pvp{li}")
    for d in range(ndt):
        fp = ps.tile([128, 2, 128], F16, tag="ft", bufs=2, name=f"fp{li}_{d}")
        for ct in range(2):
            if W == 128:
                src = y2l[ct][:, d, :]
            else:
                src = y2l[ct][:, d * rb:(d + 1) * rb, :]
            nc.tensor.transpose(fp[:, ct, :], src, idb[:, :])
        fsb = sb.tile([128, 256], F16, tag="fsb", bufs=4, name=f"fsb{li}_{d}")
        nc.scalar.copy(fsb, fp)
        nc.tensor.matmul(
            pvp[:100, :], lhsT=ptb[:, d, :], rhs=fsb[:, :],
            start=(d == 0), stop=(d == ndt - 1))
    nc.scalar.copy(att[:100, 0:256], pvp[:100, :])

    # pair exchange of [N, Z, m]
    nc.gpsimd.dma_start(out=att_in[li][:, :], in_=att[:100, :])
    nc.gpsimd.collective_compute(
        "AllGather", OP.bypass,
        ins=[att_in[li][:, :]], outs=[att_out[li][:, :]],
        replica_groups=REPLICA_GROUPS)
    ag = sb.tile([128, 2, 258], F32, tag="ag", bufs=2, name=f"ag{li}")
    nc.gpsimd.dma_start(
        out=ag[:100, :, :],
        in_=bass.AP(
            tensor=att_out[li].ap().tensor,
            offset=0,
            ap=[[258, 100], [25800, 2], [1, 258]]))
    return ag


def _attn_block_b(nc, sb, ps, tc, li, ag, ab2, idr, xt):
    # combine: out = (N0*s0 + N1*s1) / (Z0*s0 + Z1*s1), s_c = exp(m_c - M)
    mxp = sb.tile([128, 1], F32, tag="mxp", bufs=2, name=f"mxp{li}")
    nc.vector.tensor_max(mxp[:100, :], ag[:100, 0, 257:258], ag[:100, 1, 257:258])
    negM = sb.tile([128, 1], F32, tag="negM", bufs=2, name=f"negM{li}")
    nc.scalar.mul(negM[:100, :], mxp[:100, :], -1.0)
    s2 = sb.tile([128, 2], F32, tag="s2", bufs=2, name=f"s2_{li}")
    nc.scalar.activation(out=s2[:100, 0:1], in_=ag[:100, 0, 257:258],
                         func=AF.Exp, bias=negM[:100, :], scale=1.0)
    nc.scalar.activation(out=s2[:100, 1:2], in_=ag[:100, 1, 257:258],
                         func=AF.Exp, bias=negM[:100, :], scale=1.0)
    nm = sb.tile([128, 257], F32, tag="nm", bufs=2, name=f"nm{li}")
    nc.vector.tensor_scalar_mul(
        out=nm[:100, :], in0=ag[:100, 0, 0:257], scalar1=s2[:100, 0:1])
    nc.vector.scalar_tensor_tensor(
        out=nm[:100, :], in0=ag[:100, 1, 0:257], scalar=s2[:100, 1:2],
        in1=nm[:100, :], op0=OP.mult, op1=OP.add)
    rden = sb.tile([128, 1], F32, tag="rden", bufs=2, name=f"rden{li}")
    nc.vector.reciprocal(rden[:100, :], nm[:100, 256:257])
    xa = sb.tile([128, 256], F32R, tag="xa", bufs=2, name=f"xa{li}")
    nc.vector.tensor_scalar_mul(
        out=xa[:100, :], in0=nm[:100, 0:256], scalar1=rden[:100, :])
    # transpose to (C, Q) and apply GN2 affine a2*x+b2
    for ct in range(2):
        tp = ps.tile([128, 128], F32R, tag="ft", bufs=2, name=f"tp{li}_{ct}")
        nc.tensor.transpose(
            tp[:, :100], xa[:100, ct * 128:(ct + 1) * 128], idr[:100, :100])
        nc.scalar.activation(
            out=xt[:, li * 2 + ct, :], in_=tp[:, :100],
            func=AF.Identity,
            bias=ab2[:, ct, 1:2], scale=ab2[:, ct, 0:1])


def _get_nc():
    if "nc" not in _NC_CACHE:
        _NC_CACHE["nc"] = _build()
    return _NC_CACHE["nc"]


def _make_in_maps(pred_embds, feat0, feat1, feat2, proj_params, agg_params,
                  emb_params, trk_params):
    feats = [np.asarray(feat0), np.asarray(feat1), np.asarray(feat2)]
    pred_embds = np.asarray(pred_embds)

    common = {}
    for li, lv in enumerate(LEVELS):
        w1, b1, gs1, gb1, w2, b2, gs2, gb2 = [np.asarray(p) for p in proj_params[li]]
        common[f"w1t{li}"] = np.ascontiguousarray(
            w1[:, :, 0, 0].T.astype(np.float32))
        common[f"w2t{li}"] = np.ascontiguousarray(
            w2.transpose(1, 2, 3, 0).reshape(256, 9, 256)).astype(np.float16)
        common[f"b1_{li}"] = b1.reshape(256, 1).astype(np.float32)
        common[f"b2_{li}"] = b2.reshape(256, 1).astype(np.float32)
        common[f"gsb{li}"] = np.stack(
            [gs1.reshape(256, 1), gb1.reshape(256, 1),
             gs2.reshape(256, 1), gb2.reshape(256, 1)]).astype(np.float32)
    mlp_params = {"agg": agg_params, "emb": emb_params, "trk": trk_params}
    for nm, p in mlp_params.items():
        w1, b1, w2, b2 = [np.asarray(q) for q in p]
        common[f"mw_{nm}1"] = np.ascontiguousarray(w1.T.astype(np.float32))
        common[f"mb_{nm}1"] = b1.reshape(256, 1).astype(np.float32)
        common[f"mw_{nm}2"] = np.ascontiguousarray(w2.T.astype(np.float32))
        common[f"mb_{nm}2"] = b2.reshape(256, 1).astype(np.float32)
    common["g8"] = np.kron(np.eye(16, dtype=np.float32),
                           np.ones((8, 1), np.float32)) / 16.0
    common["e16"] = np.kron(np.eye(16, dtype=np.float32),
                            np.ones((1, 8), np.float32))
    common["idb"] = np.eye(128, dtype=np.float16)
    common["idr"] = np.eye(128, dtype=np.float32)

    in_maps = []
    for c in range(NCORES):
        img, h = c // 2, c % 2
        b, t = img // T, img % T
        m = dict(common)
        for li, lv in enumerate(LEVELS):
            H, W, rs, hh = lv["H"], lv["W"], lv["rs"], lv["hh"]
            xs = np.zeros((lv["cin"], rs, W), np.float32)
            start = h * hh - 1
            lo = max(0, start)
            hi = min(H, start + rs)
            xs[:, lo - start:hi - start, :] = feats[li][img, :, lo:hi, :]
            m[f"x{li}"] = xs
        m["embq"] = np.ascontiguousarray(pred_embds[b, :, t, :]).astype(np.float32)
        keep = np.empty((128, 2), np.float32)
        keep[:, 0] = 0.0 if h == 0 else 1.0   # keep top row? (0 => zero it)
        keep[:, 1] = 1.0 if h == 0 else 0.0   # keep bottom row?
        m["padkeep"] = keep
        in_maps.append(m)
    return in_maps


def kernel(pred_embds, feat0, feat1, feat2, proj_params, agg_params,
           emb_params, trk_params):
    in_maps = _make_in_maps(pred_embds, feat0, feat1, feat2, proj_params,
                            agg_params, emb_params, trk_params)
    nc = _get_nc()
    res = run_bass_kernel_spmd(nc, in_maps, core_ids=list(range(NCORES)))
    out = np.empty((B, T, Q, C), np.float32)
    for img in range(B * T):
        b, t = img // T, img % T
        out[b, t] = res.results[2 * img]["out"].T
    return out
